# revision 10
# baseline (speedup 1.0000x reference)
"""Cross-attention (GQA + RoPE) Trainium2 Bass kernel.

Sharding: 8 cores = 4 batches x 2 head-groups.
  core i -> batch b = i // 2, head-group g = i % 2
  Each core computes 8 query heads / 2 kv heads of one batch and a
  row-parallel partial of the output projection; the host sums the two
  partials per batch.

Per-core layout (all "T" tensors have head_dim / feature on partitions):
  qT   [1024, TQ]   query^T               (host-transposed)
  kvT  [1024, TKV]  key_value^T           (host-transposed)
  wq   [1024, 512]  w_q columns of this head group, head-PERMUTED so that
                    pair-tile j holds local heads (j, j+4) -> rows (0-63, 64-127).
                    This makes the Q row base (64*(h//4)) equal the K row base
                    for every head (required: matmul lhsT/rhs partition bases
                    must match the PE row placement).
  wk   [1024, 128]  w_k columns (2 kv heads)
  wv   [1024, 128]  w_v columns
  wout [512, 1024]  w_out rows, same head permutation as wq columns
  cosF [128, TKV]   rope cos stacked [c;c;c;c]   (32 rows repeated)
  sinF [128, TKV]   rope sin stacked [-s;s;-s;s]
  maskb [128, NCH]  additive kv-mask bias per 128-chunk (0 / -30000)

Algorithm per core:
  K^T = rope(wk^T @ kvT)      resident [128, TKV]   (2 kv heads stacked)
  V   = (kvT chunks)^T @ wv   resident [128, 65*NCH] per kv head, with an
                              appended ones-column per chunk (softmax denom)
  per tq block T2, per head:
     scores^T chunk [tkv 128, tq T2] = K_c^T.T @ Q^T   (PSUM)
     e = exp(0.125*scores^T + mask_bias)               (ACT, bias per partition)
     psum_o [65, T2] += V_c_aug.T @ e                  (row 64 = sum of exp)
     attnT = psum_o[0:64] * broadcast(1/psum_o[64])    (DVE + gpsimd bcast)
  out[tq, :] partial = attnT.T @ wout                  (PSUM -> DMA)
"""

import os
from contextlib import ExitStack

import numpy as np

import concourse.bass as bass
import concourse.bacc as bacc
import concourse.mybir as mybir
import concourse.tile as tile
from concourse.bass_utils import run_bass_kernel_spmd

F32 = mybir.dt.float32
R32 = mybir.dt.float32r

D_MODEL = 1024
N_HEADS = 16
NUM_KV_HEADS = 4
D_K = 64
ROPE_BASE = 10000.0
B = 4
TQ = 2048
TKV = 2048
N_CORES = 8

NEG_BIAS = -30000.0


def build_bass(tq=TQ, tkv=TKV, t2=1024, use_f32r=True):
    """Build the single-core SPMD program (same program on all 8 cores)."""
    nc = bacc.Bacc("TRN2", target_bir_lowering=False, debug=False)
    P = 128
    NKT = tkv // 512          # kv projection tiles
    NCH = tkv // 128          # attention kv chunks
    NT2 = tq // t2            # tq blocks
    NHALF = t2 // 512         # 512-wide matmul slices per tq block
    NPAIR = 4                 # head-pair tiles per core
    DT = R32 if use_f32r else F32

    qT = nc.dram_tensor("qT", [D_MODEL, tq], DT, kind="ExternalInput").ap()
    kvT = nc.dram_tensor("kvT", [D_MODEL, tkv], DT, kind="ExternalInput").ap()
    wq = nc.dram_tensor("wq", [D_MODEL, 512], DT, kind="ExternalInput").ap()
    wk = nc.dram_tensor("wk", [D_MODEL, 128], DT, kind="ExternalInput").ap()
    wv = nc.dram_tensor("wv", [D_MODEL, 128], DT, kind="ExternalInput").ap()
    wout = nc.dram_tensor("wout", [512, D_MODEL], DT, kind="ExternalInput").ap()
    cosF = nc.dram_tensor("cosF", [P, tkv], F32, kind="ExternalInput").ap()
    sinF = nc.dram_tensor("sinF", [P, tkv], F32, kind="ExternalInput").ap()
    maskb = nc.dram_tensor("maskb", [P, NCH], F32, kind="ExternalInput").ap()
    onesc = nc.dram_tensor("onesc", [P, 64], DT, kind="ExternalInput").ap()
    out = nc.dram_tensor("out", [tq, D_MODEL], F32, kind="ExternalOutput").ap()

    with tile.TileContext(nc) as tc, ExitStack() as ctx:
        const = ctx.enter_context(tc.tile_pool(name="const", bufs=1))
        blkp = ctx.enter_context(tc.tile_pool(name="blkp", bufs=2))
        qpool = ctx.enter_context(tc.tile_pool(name="qpool", bufs=1))
        apool = ctx.enter_context(tc.tile_pool(name="apool", bufs=1))
        workp = ctx.enter_context(tc.tile_pool(name="workp", bufs=3))
        ropep = ctx.enter_context(tc.tile_pool(name="ropep", bufs=4))
        invp = ctx.enter_context(tc.tile_pool(name="invp", bufs=2))
        outp = ctx.enter_context(tc.tile_pool(name="outp", bufs=3))
        pp_big = ctx.enter_context(tc.tile_pool(name="pp_big", bufs=2, space="PSUM"))
        pp_acc = ctx.enter_context(tc.tile_pool(name="pp_acc", bufs=1, space="PSUM"))
        pp_sm = ctx.enter_context(tc.tile_pool(name="pp_sm", bufs=2, space="PSUM"))

        def MM(out_ap, lhsT, rhs, start, stop):
            nc.tensor.matmul(out_ap, lhsT, rhs, start=start, stop=stop)

        # ---- constants / weights -------------------------------------------------
        wq_sb = const.tile([P, 8, 512], DT)
        nc.gpsimd.dma_start(out=wq_sb, in_=wq.rearrange("(c p) f -> p c f", p=P))
        wk_sb = const.tile([P, 8, 128], DT)
        nc.gpsimd.dma_start(out=wk_sb, in_=wk.rearrange("(c p) f -> p c f", p=P))
        wv_sb = const.tile([P, 8, 128], DT)
        nc.gpsimd.dma_start(out=wv_sb, in_=wv.rearrange("(c p) f -> p c f", p=P))
        wout_sb = const.tile([P, 4, D_MODEL], DT)
        nc.gpsimd.dma_start(out=wout_sb, in_=wout.rearrange("(c p) f -> p c f", p=P))
        cos_sb = const.tile([P, tkv], F32)
        nc.gpsimd.dma_start(out=cos_sb, in_=cosF)
        sin_sb = const.tile([P, tkv], F32)
        nc.gpsimd.dma_start(out=sin_sb, in_=sinF)
        mask_sb = const.tile([P, NCH], F32)
        nc.gpsimd.dma_start(out=mask_sb, in_=maskb)

        Kt = const.tile([P, tkv], DT)
        Vt = [const.tile([P, NCH * 65], DT, name=f"Vt{i}") for i in range(2)]
        for i in range(2):
            nc.gpsimd.dma_start(
                out=Vt[i].rearrange("p (c k) -> p c k", k=65)[:, :, 64],
                in_=onesc[:, :NCH],
            )
        ones_sb = const.tile([1, 64], DT)
        nc.gpsimd.dma_start(out=ones_sb, in_=onesc[0:1, :])

        def rope_apply(dest, ps, col0, width):
            """dest[128, width] (SBUF) = rope(ps[128, width] PSUM), positions
            col0..col0+width. Rows are two stacked heads, each [x1(32); x2(32)]."""
            cs = cos_sb[:, col0 : col0 + width]
            t_cos = ropep.tile([P, t2], F32, tag="rope", name="t_cos")
            t_u = ropep.tile([P, t2], F32, tag="rope", name="t_u")
            tc_ = t_cos[:, :width]
            tu_ = t_u[:, :width]
            nc.vector.tensor_mul(tc_, ps, cs)
            for b0 in (0, 64):
                # sinF rows [b0:b0+32] = -sin, [b0+32:b0+64] = +sin
                nc.vector.tensor_mul(
                    tu_[b0 : b0 + 32, :],
                    ps[b0 + 32 : b0 + 64, :],
                    sin_sb[b0 : b0 + 32, col0 : col0 + width],
                )
                nc.vector.tensor_mul(
                    tu_[b0 + 32 : b0 + 64, :],
                    ps[b0 : b0 + 32, :],
                    sin_sb[b0 + 32 : b0 + 64, col0 : col0 + width],
                )
            nc.vector.tensor_add(dest, tc_, tu_)

        # ---- phase KV: K/V projections ------------------------------------------
        for kt in range(NKT):
            kv_blk = blkp.tile([P, 8, 512], DT, tag="blk", name="kv_blk")
            nc.gpsimd.dma_start(
                out=kv_blk,
                in_=kvT.rearrange("(c p) t -> p c t", p=P)[
                    :, :, kt * 512 : (kt + 1) * 512
                ],
            )
            ps_k = pp_sm.tile([P, 512], F32, tag="sm", name="ps_k")
            for d in range(8):
                MM(ps_k, wk_sb[:, d, :], kv_blk[:, d, :], d == 0, d == 7)
            rope_apply(Kt[:, kt * 512 : (kt + 1) * 512], ps_k, kt * 512, 512)
            for s in range(4):
                ps_v = pp_sm.tile([P, 512], F32, tag="sm", name="ps_v")
                pv = ps_v[:, 0:128]
                for d in range(8):
                    MM(
                        pv,
                        kv_blk[:, d, s * 128 : (s + 1) * 128],
                        wv_sb[:, d, :],
                        d == 0,
                        d == 7,
                    )
                c = kt * 4 + s
                nc.vector.tensor_copy(
                    out=Vt[0][:, c * 65 : c * 65 + 64], in_=pv[:, 0:64]
                )
                nc.vector.tensor_copy(
                    out=Vt[1][:, c * 65 : c * 65 + 64], in_=pv[:, 64:128]
                )

        # ---- per tq block: Q proj -> attention -> output projection -------------
        for it2 in range(NT2):
            q_blks = []
            for half in range(NHALF):
                qb = blkp.tile([P, 8, 512], DT, tag="blk", name="q_blk")
                c0 = it2 * t2 + half * 512
                nc.gpsimd.dma_start(
                    out=qb,
                    in_=qT.rearrange("(c p) t -> p c t", p=P)[:, :, c0 : c0 + 512],
                )
                q_blks.append(qb)

            Qt = []
            for j in range(NPAIR):
                ps_q = pp_big.tile([P, t2], F32, tag="big", name="ps_q")
                for half in range(NHALF):
                    for d in range(8):
                        MM(
                            ps_q[:, half * 512 : (half + 1) * 512],
                            wq_sb[:, d, j * 128 : (j + 1) * 128],
                            q_blks[half][:, d, :],
                            d == 0,
                            d == 7,
                        )
                qt = qpool.tile([P, t2], DT, tag=f"Q{j}", name=f"Qt{j}")
                rope_apply(qt, ps_q, it2 * t2, t2)
                Qt.append(qt)

            attnT = [
                apool.tile([P, t2], DT, tag=f"A{j}", name=f"attnT{j}")
                for j in range(NPAIR)
            ]

            for h in range(8):
                j = h % 4
                kvh = h // 4
                base = 64 * kvh
                ps_o = pp_acc.tile([65, t2], F32, tag="acc", name="ps_o")
                for c in range(NCH):
                    ps_s = pp_big.tile([P, t2], F32, tag="big", name="ps_s")
                    for half in range(NHALF):
                        MM(
                            ps_s[:, half * 512 : (half + 1) * 512],
                            Kt[base : base + 64, c * 128 : (c + 1) * 128],
                            Qt[j][base : base + 64, half * 512 : (half + 1) * 512],
                            True,
                            True,
                        )
                    ex = workp.tile([P, t2], DT, tag="expT", name="ex")
                    nc.scalar.activation(
                        out=ex,
                        in_=ps_s,
                        func=mybir.ActivationFunctionType.Exp,
                        bias=mask_sb[:, c : c + 1],
                        scale=0.125,
                    )
                    for half in range(NHALF):
                        MM(
                            ps_o[:, half * 512 : (half + 1) * 512],
                            Vt[kvh][:, c * 65 : c * 65 + 65],
                            ex[:, half * 512 : (half + 1) * 512],
                            c == 0,
                            c == NCH - 1,
                        )
                # copy PSUM out fast to release the accumulator slot (keeps PE
                # fed -> HAM stays warm); normalize off the critical path.
                U = workp.tile([65, t2], F32, tag="unorm", name="U")
                nc.vector.tensor_copy(out=U, in_=ps_o)
                inv = invp.tile([1, t2], DT, tag="inv", name="inv")
                with nc.allow_low_precision("f32r denominators feed the bcast matmul"):
                    nc.vector.reciprocal(out=inv, in_=U[64:65, :])
                for half in range(NHALF):
                    hs = slice(half * 512, (half + 1) * 512)
                    ps_b = pp_sm.tile([64, 512], F32, tag="sm", name="ps_b")
                    MM(ps_b, ones_sb, inv[:, hs], True, True)
                    invb = invp.tile([64, 512], F32, tag="invb", name="invb")
                    nc.vector.tensor_copy(out=invb, in_=ps_b)
                    nc.vector.tensor_mul(
                        attnT[j][base : base + 64, hs], U[0:64, hs], invb
                    )

            for s in range(t2 // 128):
                ob = outp.tile([P, D_MODEL], F32, tag="ob", name="ob")
                for n in range(2):
                    ps_f = pp_sm.tile([P, 512], F32, tag="sm", name="ps_f")
                    for p_ in range(NPAIR):
                        MM(
                            ps_f,
                            attnT[p_][:, s * 128 : (s + 1) * 128],
                            wout_sb[:, p_, n * 512 : (n + 1) * 512],
                            p_ == 0,
                            p_ == NPAIR - 1,
                        )
                    nc.vector.tensor_copy(
                        out=ob[:, n * 512 : (n + 1) * 512], in_=ps_f
                    )
                r0 = it2 * t2 + s * 128
                nc.sync.dma_start(out=out[r0 : r0 + 128, :], in_=ob)

    nc.compile()
    return nc


# ---------------------------------------------------------------------------
# host-side sharding / prep
# ---------------------------------------------------------------------------

_HEAD_PERM = [0, 4, 1, 5, 2, 6, 3, 7]  # local head order inside pair tiles


def _rope_tables(tkv):
    theta = ROPE_BASE ** (-np.arange(0, D_K, 2, dtype=np.float32) / D_K)  # [32]
    pos = np.arange(tkv, dtype=np.float32)[:, None]  # [tkv,1]
    ang = pos * theta[None, :]  # [tkv,32]
    c = np.cos(ang).T.astype(np.float32)  # [32, tkv]
    s = np.sin(ang).T.astype(np.float32)
    cosF = np.concatenate([c, c, c, c], axis=0)
    sinF = np.concatenate([-s, s, -s, s], axis=0)
    return np.ascontiguousarray(cosF), np.ascontiguousarray(sinF)


def make_in_maps(query, key_value, kv_mask, w_q, w_k, w_v, w_out, tq=TQ, tkv=TKV):
    nb = query.shape[0]
    cosF, sinF = _rope_tables(max(tq, tkv))
    cosF = cosF[:, :tkv] if cosF.shape[1] != tkv else cosF
    sinF = sinF[:, :tkv] if sinF.shape[1] != tkv else sinF
    cosQ = cosF  # same tables sliced by column inside the kernel
    del cosQ
    in_maps = []
    col_perm = np.concatenate(
        [np.arange(h * D_K, (h + 1) * D_K) for h in _HEAD_PERM]
    )
    for core in range(2 * nb):
        b = core // 2
        g = core % 2
        qTb = np.ascontiguousarray(query[b].T.astype(np.float32))
        kvTb = np.ascontiguousarray(key_value[b].T.astype(np.float32))
        wq_g = w_q[:, g * 512 : (g + 1) * 512][:, col_perm]
        wk_g = w_k[:, g * 128 : (g + 1) * 128]
        wv_g = w_v[:, g * 128 : (g + 1) * 128]
        wout_g = w_out[g * 512 : (g + 1) * 512, :][col_perm, :]
        maskb = np.where(kv_mask[b], 0.0, NEG_BIAS).astype(np.float32)
        maskb = np.ascontiguousarray(maskb.reshape(tkv // 128, 128).T)
        ones_arr = np.ones((128, 64), np.float32)
        in_maps.append(
            {
                "qT": qTb,
                "kvT": kvTb,
                "wq": np.ascontiguousarray(wq_g.astype(np.float32)),
                "wk": np.ascontiguousarray(wk_g.astype(np.float32)),
                "wv": np.ascontiguousarray(wv_g.astype(np.float32)),
                "wout": np.ascontiguousarray(wout_g.astype(np.float32)),
                "cosF": cosF,
                "sinF": sinF,
                "maskb": maskb,
                "onesc": ones_arr,
            }
        )
    return in_maps


_NC_CACHE = {}


def _get_nc(tq=TQ, tkv=TKV, t2=1024, use_f32r=True):
    key = (tq, tkv, t2, use_f32r)
    if key not in _NC_CACHE:
        _NC_CACHE[key] = build_bass(tq, tkv, t2, use_f32r)
    return _NC_CACHE[key]


def _run(inputs, trace=False):
    query = np.asarray(inputs["query"], dtype=np.float32)
    key_value = np.asarray(inputs["key_value"], dtype=np.float32)
    kv_mask = np.asarray(inputs["kv_mask"])
    w_q = np.asarray(inputs["w_q"], dtype=np.float32)
    w_k = np.asarray(inputs["w_k"], dtype=np.float32)
    w_v = np.asarray(inputs["w_v"], dtype=np.float32)
    w_out = np.asarray(inputs["w_out"], dtype=np.float32)
    nb, tq, _ = query.shape
    tkv = key_value.shape[1]

    nc = _get_nc(tq, tkv)
    in_maps = make_in_maps(query, key_value, kv_mask, w_q, w_k, w_v, w_out, tq, tkv)
    res = run_bass_kernel_spmd(
        nc, in_maps, list(range(2 * nb)), trace=trace, trace_cores=[0]
    )
    outs = [np.asarray(r["out"]) for r in res.results]
    full = np.stack([outs[2 * b] + outs[2 * b + 1] for b in range(nb)])

    query_mask = np.asarray(inputs["query_mask"])
    if not query_mask.all():
        # masked query rows: reference yields uniform attention over all kv
        for b in range(nb):
            rows = ~query_mask[b]
            if rows.any():
                V = key_value[b] @ w_v  # [tkv, 256]
                meanV = V.mean(axis=0)  # [256]
                group = N_HEADS // NUM_KV_HEADS
                feat = np.concatenate([meanV.reshape(NUM_KV_HEADS, D_K)[h // group]
                                       for h in range(N_HEADS)])
                full[b, rows, :] = feat @ w_out
    return full.astype(np.float32), res


def kernel(**inputs):
    out, _ = _run(inputs, trace=False)
    return out


def kernel_traced(**inputs):
    out, res = _run(inputs, trace=True)
    return out, res


if __name__ == "__main__":
    print("kernel.py is a library; use test.py")
